# revision 43
# baseline (speedup 1.0000x reference)
"""AttentionBlock kernel for 8x Trainium2 NeuronCores.

Strategy: data-parallel over batch (B=8 -> 1 batch element per core), with
fp8-e4m3 DoubleRow matmuls (K=256 contraction per instruction = 2x the
FLOPs/column of bf16) everywhere the numerics allow, bf16 only for the
score matmuls (exp amplifies q/k quantization noise; fp8 scores fail the
2e-2 gate, bf16 scores measure ~1.3e-2 in simulation).

Per-core layout (channel-major, "transposed", no on-chip transposes):

  x8  [128, 2, 1024] fp8 pairs   (contraction rows c = i*256 + s*128 + p)
  wp8 [128, 2, 1536] fp8 pairs   (host-reordered columns [Q|K|V], x16 scale)
  A:  q/k^T[d, tok]  = wp8^T x8 (fp8 DR), DVE adds 16*bq to q (k-bias
      cancels in softmax and is dropped)
  B:  v[tok, och]    = x8^T wp8_V (fp8 DR), Pool copies psum -> v8 pairs
  S:  S^T[j, i]      = kT^T qT per j-tile (bf16)
  E = exp(S * scale/256 - 4) on ScalarE, fp8 out, pair tiles (the -4 bias
      keeps e^s under fp8-e4m3 max 240 and cancels in softmax)
  AV: O^T[d, i]     += v8^T E (fp8 DR); CS: colsums via banded-ones fp8 DR
      matmuls into a [2, 512] psum tile per head
  recip (DVE f32r), broadcast via u-band matmul (rows scaled 1/16), norm:
      Pool copies O psum -> bf16, DVE multiplies by broadcast recip -> fp8
  D:  out^T[c, tok]  = wo8^T oT8 (fp8 DR) + (bres + x) via one STT op,
      where bres = bo + Wo^T bv is precomputed on the host (sum_j P = 1).

Schedule: ScalarE runs the 32 exp ops back to back (~30us); the PE stream
is emitted so scores of head h+1 and AV/CS of head h interleave inside
head h's exp window.  All input DMAs trigger from the Pool queue (cheap
descriptor config), x8 from the SP queue, outputs from SP at the tail.
"""

import sys

sys.path.insert(0, "/opt/trn_rl_repo")

import numpy as np
import ml_dtypes

import concourse.bass as bass
import concourse.tile as tile
import concourse.mybir as mybir
from concourse.bass_utils import run_bass_kernel_spmd

B, C, HW = 8, 512, 1024
NH, DK = 4, 128
SCALE = float(DK) ** -0.5
WPS = 16.0  # host pre-scale of Wp into fp8 normal range
EXP_BIAS = -4.0

F32 = mybir.dt.float32
F32R = mybir.dt.float32r
BF16 = mybir.dt.bfloat16
F8 = mybir.dt.float8e4
DR = mybir.MatmulPerfMode.DoubleRow

# ---------------------------------------------------------------------------
# Walrus in this container supports only ONE embedded sync-wait per
# instruction; Tile emits multi-wait instructions, so rewrite each into
# single-wait NoOps + the instruction keeping its last wait.
# ---------------------------------------------------------------------------
_wsplit_counter = [0]


def _split_multi_waits(nc):
    for fn in nc.m.functions:
        for blk in fn.blocks:
            insts = blk.instructions
            if not insts:
                continue
            new = []
            changed = False
            for inst in insts:
                si = inst.sync_info
                waits = list(si.on_wait) if si is not None and si.on_wait else []
                if len(waits) > 1:
                    changed = True
                    for w in waits[:-1]:
                        _wsplit_counter[0] += 1
                        nop = mybir.InstNoOp(
                            name=f"WSPLIT-{_wsplit_counter[0]}",
                            ins=[],
                            outs=[],
                            engine=inst.engine,
                        )
                        nop.sync_info = mybir.SyncInfo(on_wait=[w], on_update=[])
                        nc.register_instruction(nop, overwrite=True)
                        new.append(nop)
                    inst.sync_info = mybir.SyncInfo(
                        on_wait=[waits[-1]], on_update=list(si.on_update or [])
                    )
                new.append(inst)
            if changed:
                blk.instructions = new


def build_attention_nc():
    nc = bass.Bass("TRN2")
    # partition-major host layouts: one contiguous DMA per tensor chunk
    x8d = nc.dram_tensor("x8", [128, 4 * HW], F8, kind="ExternalInput")
    x16d = nc.dram_tensor("x16", [128, 4 * HW], BF16, kind="ExternalInput")
    # wp block-major: [p, (block q/k/v, c-subtile, 512)]
    wpd = nc.dram_tensor("wp", [128, 3 * 4 * 512], F8, kind="ExternalInput")
    wod = nc.dram_tensor("wo", [128, 4 * C], F8, kind="ExternalInput")
    bqd = nc.dram_tensor("bq", [128, 4], F32, kind="ExternalInput")
    bresd = nc.dram_tensor("bres", [128, 4], F32, kind="ExternalInput")
    t2d = nc.dram_tensor("t2", [128, 512], F8, kind="ExternalInput")
    u2d = nc.dram_tensor("u2", [2, 256], F32R, kind="ExternalInput")
    outd = nc.dram_tensor("out", [C, HW], F32, kind="ExternalOutput")

    x8d, x16d, wpd, wod, bqd, bresd, t2d, u2d, outd = (
        t.ap() for t in (x8d, x16d, wpd, wod, bqd, bresd, t2d, u2d, outd)
    )

    EXP = mybir.ActivationFunctionType.Exp
    ADD = mybir.AluOpType.add
    MUL = mybir.AluOpType.mult
    IC = [slice(0, 512), slice(512, 1024)]

    with tile.TileContext(nc) as tc:
        with (
            tc.tile_pool(name="persist", bufs=1) as persist,
            tc.tile_pool(name="epool", bufs=8) as epool,
            tc.tile_pool(name="o16p", bufs=4) as o16p,
            tc.tile_pool(name="outp", bufs=2) as outp,
            tc.tile_pool(name="psM", bufs=2, space="PSUM") as psM,
            tc.tile_pool(name="psX", bufs=4, space="PSUM") as psX,
        ):
            # ---- persistent SBUF tensors -------------------------------
            x8_sb = persist.tile([128, 4 * HW], F8, tag="x8", name="x8")
            x16_sb = persist.tile([128, 4 * HW], BF16, tag="x16", name="x16")
            wp_sb = [
                persist.tile([128, 2048], F8, tag=f"wp{b}", name=f"wp{b}")
                for b in range(3)
            ]
            wo_sb = persist.tile([128, 4 * C], F8, tag="wo", name="wo")
            qk_sb = [persist.tile([128, HW], BF16, tag=f"qk{i}", name=f"qk{i}") for i in range(8)]
            v8_sb = [persist.tile([128, 1024], F8, tag=f"v{i}", name=f"v{i}") for i in range(4)]
            oT8_sb = [persist.tile([128, 2048], F8, tag=f"oT{i}", name=f"oT{i}") for i in range(2)]
            bq_sb = persist.tile([128, 4], F32, tag="bq", name="bq")
            bres_sb = persist.tile([128, 4], F32, tag="br", name="br")
            csr_sb = [persist.tile([2, 512], F32R, tag=f"cs{i}", name=f"cs{i}") for i in range(4)]
            t2_sb = persist.tile([128, 512], F8, tag="t2", name="t2")
            u2_sb = persist.tile([2, 256], F32R, tag="u2", name="u2")
            warm_sb = persist.tile([1, 2], F32, tag="warm", name="warm")
            ebias_sb = persist.tile([128, 1], F32, tag="ebias", name="ebias")

            x84 = x8_sb[:].rearrange("p (c n) -> p c n", c=4)
            # per-block [p, c-subtile, 512] views of wp
            wpb = [wp_sb[b][:].rearrange("p (c n) -> p c n", c=4) for b in range(3)]
            wo4 = wo_sb[:].rearrange("p (c n) -> p c n", c=4)
            v83 = [t[:].rearrange("p (s n) -> p s n", s=2) for t in v8_sb]
            oT83 = [t[:].rearrange("p (s n) -> p s n", s=2) for t in oT8_sb]
            t23 = [
                t2_sb[:, 0:256].rearrange("p (s n) -> p s n", s=2),
                t2_sb[:, 256:512].rearrange("p (s n) -> p s n", s=2),
            ]

            # ---- loads: few big DMAs (each completion sem on the
            # critical path costs ~1us; ring FIFO follows trigger order) --
            nc.sync.dma_start(out=x8_sb[:, 0:2048], in_=x8d[:, 0:2048])
            nc.sync.dma_start(out=x8_sb[:, 2048:4096], in_=x8d[:, 2048:4096])
            for b in range(3):
                nc.sync.dma_start(
                    out=wp_sb[b], in_=wpd[:, b * 2048 : (b + 1) * 2048]
                )
            nc.scalar.dma_start(out=bq_sb, in_=bqd[:, :])
            nc.scalar.dma_start(out=t2_sb, in_=t2d[:, :])
            nc.scalar.dma_start(out=u2_sb, in_=u2d[:, :])
            nc.scalar.dma_start(out=bres_sb, in_=bresd[:, :])
            # bulk needed only at the tail: held behind k0 so its descriptors
            # stay off the rings during the startup transfers
            dgate = persist.tile([1, 1], BF16, tag="dgate", name="dgate")
            nc.gpsimd.tensor_copy(out=dgate[:], in_=qk_sb[1][0:1, 0:1])
            nc.gpsimd.dma_start(out=x16_sb[:, 0:2048], in_=x16d[:, 0:2048])
            nc.gpsimd.dma_start(out=x16_sb[:, 2048:4096], in_=x16d[:, 2048:4096])
            nc.gpsimd.dma_start(out=wo_sb, in_=wod[:, :])

            # ---- preload the Exp table while DMAs land -----------------
            nc.vector.memset(warm_sb[:], 0.0)
            nc.vector.memset(ebias_sb[:], EXP_BIAS)
            nc.scalar.activation(
                out=warm_sb[0:1, 0:1],
                in_=warm_sb[0:1, 1:2],
                func=EXP,
                bias=ebias_sb[0:1],
            )

            # ---- helpers ----------------------------------------------
            def proj_qk(h, qk):
                """q/k^T [128 d, 1024] per-ic psum halves (fp8 DR) + DVE cast."""
                for ic in range(2):
                    ps = psX.tile([128, 512], F32, tag="psX", name="psX")
                    for i in range(2):
                        nc.tensor.matmul(
                            ps[:],
                            wpb[qk][:, 2 * i : 2 * i + 2, h * 128 : (h + 1) * 128],
                            x84[:, 2 * i : 2 * i + 2, IC[ic]],
                            start=(i == 0),
                            stop=(i == 1),
                            perf_mode=DR,
                            skip_group_check=True,
                        )
                    if qk == 0:
                        nc.vector.tensor_scalar_add(
                            out=qk_sb[h * 2][:, IC[ic]],
                            in0=ps[:],
                            scalar1=bq_sb[:, h : h + 1],
                        )
                    else:
                        nc.vector.tensor_copy(out=qk_sb[h * 2 + 1][:, IC[ic]], in_=ps[:])

            def proj_v(jt):
                """v [tok, och] one j-tile (fp8 DR) + Act copy to fp8."""
                ps = psX.tile([128, 512], F32, tag="psX", name="psX")
                for i in range(2):
                    nc.tensor.matmul(
                        ps[:],
                        x84[:, 2 * i : 2 * i + 2, jt * 128 : (jt + 1) * 128],
                        wpb[2][:, 2 * i : 2 * i + 2, :],
                        start=(i == 0),
                        stop=(i == 1),
                        perf_mode=DR,
                        skip_group_check=True,
                    )
                nc.scalar.copy(
                    out=v8_sb[jt // 2][:, (jt % 2) * 512 : (jt % 2 + 1) * 512],
                    in_=ps[:],
                )

            e_tiles = {}

            def scores(h, jt):
                """S^T [128 j, 1024 i] (bf16) + exp -> E fp8 pair slot."""
                qT, kT = qk_sb[h * 2], qk_sb[h * 2 + 1]
                ps = psM.tile([128, HW], F32, tag="psM", name="psM")
                for ic in range(2):
                    nc.tensor.matmul(
                        ps[:, IC[ic]],
                        kT[:, jt * 128 : (jt + 1) * 128],
                        qT[:, IC[ic]],
                    )
                jp, s = jt // 2, jt % 2
                if s == 0:
                    e_tiles[(h, jp)] = epool.tile([128, 2048], F8, tag="E", name="E")
                nc.scalar.activation(
                    out=e_tiles[(h, jp)][:, s * 1024 : (s + 1) * 1024],
                    in_=ps[:],
                    func=EXP,
                    scale=SCALE / (WPS * WPS),
                    bias=ebias_sb[:],
                )

            def e3(h, jp):
                return e_tiles[(h, jp)][:].rearrange("p (s n) -> p s n", s=2)

            av_ps = {}  # (h, ic) -> psX tile
            cs_ps = {}  # h -> psX tile

            def av_alloc(h, ic):
                av_ps[(h, ic)] = psX.tile([128, 512], F32, tag="psX", name="psX")

            def av_mm(h, jp, ic):
                if (h, ic) not in av_ps:
                    av_alloc(h, ic)
                nc.tensor.matmul(
                    av_ps[(h, ic)][:],
                    v83[jp][:, :, h * 128 : (h + 1) * 128],
                    e3(h, jp)[:, :, IC[ic]],
                    start=(jp == 0),
                    stop=(jp == 3),
                    perf_mode=DR,
                    skip_group_check=True,
                )

            def cs_mm(h, jp):
                if jp == 0:
                    cs_ps[h] = psX.tile([128, 512], F32, tag="psX", name="psX")
                for ic in range(2):
                    nc.tensor.matmul(
                        cs_ps[h][:],
                        t23[ic],
                        e3(h, jp)[:, :, IC[ic]],
                        start=(jp == 0 and ic == 0),
                        stop=(jp == 3 and ic == 1),
                        perf_mode=DR,
                        skip_group_check=True,
                    )

            o16_tiles = {}

            def o_copy(h, ic):
                o16 = o16p.tile([128, 512], BF16, tag="o16", name="o16")
                nc.vector.tensor_copy(out=o16[:], in_=av_ps[(h, ic)][:])
                o16_tiles[(h, ic)] = o16

            def recip(h):
                with nc.allow_low_precision(reason="softmax denom recip f32r"):
                    nc.vector.reciprocal(out=csr_sb[h][:], in_=cs_ps[h][0:2, :])

            def bc_norm(h, ic):
                bc = psX.tile([128, 512], F32, tag="psX", name="psX")
                nc.tensor.matmul(
                    bc[:],
                    u2_sb[:, (1 - ic) * 128 : (2 - ic) * 128],
                    csr_sb[h][:],
                )
                nc.vector.tensor_tensor(
                    out=oT8_sb[h // 2][
                        :, (h % 2) * 1024 + ic * 512 : (h % 2) * 1024 + (ic + 1) * 512
                    ],
                    in0=o16_tiles[(h, ic)][:],
                    in1=bc[:],
                    op=MUL,
                )

            d_ps = {}

            def proj_out_mm(kc, mlist):
                if kc not in d_ps:
                    d_ps[kc] = psM.tile([128, HW], F32, tag="psM", name="psM")
                ps = d_ps[kc]
                for m in mlist:
                    for ic in range(2):
                        nc.tensor.matmul(
                            ps[:, IC[ic]],
                            wo4[:, 2 * m : 2 * m + 2, kc * 128 : (kc + 1) * 128],
                            oT83[m][:, :, IC[ic]],
                            start=(m == 0),
                            stop=(m == 1),
                            perf_mode=DR,
                            skip_group_check=True,
                        )

            def proj_out_head(kc, h, stop):
                """single-head (K=128, non-DR fp8) contribution to d_ps[kc]."""
                ps = d_ps[kc]
                for ic in range(2):
                    nc.tensor.matmul(
                        ps[:, IC[ic]],
                        wo_sb[:, h * 512 + kc * 128 : h * 512 + (kc + 1) * 128],
                        oT8_sb[h // 2][
                            :, (h % 2) * 1024 + ic * 512 : (h % 2) * 1024 + (ic + 1) * 512
                        ],
                        start=False,
                        stop=stop,
                        skip_group_check=True,
                    )

            def finish(kc):
                ps = d_ps[kc]
                ot = outp.tile([128, HW], F32, tag="out", name="out")
                nc.vector.scalar_tensor_tensor(
                    out=ot[:],
                    in0=ps[:],
                    scalar=bres_sb[:, kc : kc + 1],
                    in1=x16_sb[:, kc * 1024 : (kc + 1) * 1024],
                    op0=ADD,
                    op1=ADD,
                )
                nc.sync.dma_start(out=outd[kc * 128 : (kc + 1) * 128, :], in_=ot[:])

            # ---- prologue ---------------------------------------------
            proj_qk(0, 0)
            proj_qk(0, 1)
            scores(0, 0)
            scores(0, 1)
            proj_qk(1, 0)
            scores(0, 2)
            scores(0, 3)
            proj_qk(1, 1)
            proj_v(0)
            proj_v(1)
            scores(0, 4)
            scores(0, 5)
            proj_v(2)
            proj_v(3)
            scores(0, 6)
            scores(0, 7)
            proj_v(4)
            proj_v(5)
            proj_v(6)
            proj_v(7)

            # ---- head loop --------------------------------------------
            # trailing ops of head h-1 (which wait the last exps of head
            # h-1) are emitted at the START of iter h so they never block
            # the next head's score stream; the psX 4-slot rotation order
            # is [AVic0, AVic1, CS] + bc pairs, arranged so every slot
            # reuse waits on a fast o_copy/cast, never the slow recip.
            for h in range(NH):
                if h < 3:
                    scores(h + 1, 0)
                    scores(h + 1, 1)
                if h >= 1:
                    cs_mm(h - 1, 3)
                    av_mm(h - 1, 3, 1)
                    o_copy(h - 1, 1)
                    recip(h - 1)
                if h == 0:
                    proj_qk(2, 0)
                    proj_qk(2, 1)
                if h < 3:
                    scores(h + 1, 2)
                    scores(h + 1, 3)
                if h == 0:
                    proj_qk(3, 0)
                    proj_qk(3, 1)
                if h >= 2:
                    bc_norm(h - 2, 0)
                    bc_norm(h - 2, 1)
                av_mm(h, 0, 0)
                av_alloc(h, 1)
                av_mm(h, 1, 0)
                cs_mm(h, 0)
                cs_mm(h, 1)
                if h < 3:
                    scores(h + 1, 4)
                    scores(h + 1, 5)
                cs_mm(h, 2)
                av_mm(h, 2, 0)
                av_mm(h, 0, 1)
                av_mm(h, 1, 1)
                if h == 3:
                    proj_out_mm(0, [0])
                    proj_out_mm(1, [0])
                if h < 3:
                    scores(h + 1, 6)
                    scores(h + 1, 7)
                av_mm(h, 3, 0)
                av_mm(h, 2, 1)
                o_copy(h, 0)

            # ---- tail -------------------------------------------------
            cs_mm(3, 3)
            av_mm(3, 3, 1)
            o_copy(3, 1)
            recip(3)
            bc_norm(2, 0)
            bc_norm(2, 1)
            proj_out_head(0, 2, stop=False)
            proj_out_head(1, 2, stop=False)
            bc_norm(3, 0)
            bc_norm(3, 1)
            proj_out_head(0, 3, stop=True)
            proj_out_head(1, 3, stop=True)
            finish(0)
            finish(1)
            proj_out_mm(2, [0, 1])
            finish(2)
            proj_out_mm(3, [0, 1])
            finish(3)

    _split_multi_waits(nc)
    return nc


_NC_CACHE = {}


def _get_nc():
    if "nc" not in _NC_CACHE:
        _NC_CACHE["nc"] = build_attention_nc()
    return _NC_CACHE["nc"]


def _host_consts():
    f8 = ml_dtypes.float8_e4m3
    # selector weights [p, (half, s, 128)]: half ic has its column ic set,
    # in both k-subtile slots, so matmul output row ic gets the colsum
    t2 = np.zeros((128, 512), dtype=f8)
    for s in range(2):
        t2[:, s * 128 + 0] = 1.0
        t2[:, 256 + s * 128 + 1] = 1.0
    u2 = np.zeros((2, 256), dtype=np.float32)
    u2[0, 128:256] = 1.0 / WPS
    u2[1, 0:128] = 1.0 / WPS
    return t2, u2


def _prep_weights(Wp, bp, Wo, bo):
    """Reorder Wp/bp columns to [Q|K|V] blocks, pre-scale by WPS, cast fp8;
    fold the v bias through the output projection on the host."""
    f8 = ml_dtypes.float8_e4m3
    Wp = np.ascontiguousarray(Wp, dtype=np.float32)
    bp = np.ascontiguousarray(bp, dtype=np.float32).reshape(-1)
    Wo = np.ascontiguousarray(Wo, dtype=np.float32)
    bo = np.ascontiguousarray(bo, dtype=np.float32).reshape(-1)
    qcols = np.concatenate([np.arange(h * 384, h * 384 + 128) for h in range(NH)])
    kcols = qcols + 128
    vcols = qcols + 256
    order = np.concatenate([qcols, kcols, vcols])
    wp8 = (Wp[:, order] * WPS).astype(f8)
    bq = (bp[qcols] * WPS).astype(np.float32).reshape(C, 1)
    bres = (bo + Wo.T @ bp[vcols]).astype(np.float32).reshape(C, 1)
    wo8 = Wo.astype(f8)
    return wp8, bq, wo8, bres


def _pmajor(a):
    """[512, N] -> [128, 4*N]: row c = cs*128 + p lands at [p, cs*N + n]."""
    n = a.shape[1]
    return np.ascontiguousarray(a.reshape(4, 128, n).transpose(1, 0, 2).reshape(128, 4 * n))


def run_sharded(x, Wp, bp, Wo, bo, **spmd_kwargs):
    """Shard over batch, run on cores 0-7, gather.  Returns ([B,C,H,W], res)."""
    f8 = ml_dtypes.float8_e4m3
    x = np.ascontiguousarray(x, dtype=np.float32).reshape(B, C, HW)
    wp8, bq, wo8, bres = _prep_weights(Wp, bp, Wo, bo)
    # block-major: [p, (block, c-subtile, 512)]
    wp8 = np.ascontiguousarray(
        wp8.reshape(4, 128, 3, 512).transpose(1, 2, 0, 3).reshape(128, 6144)
    )
    wo8 = _pmajor(wo8)
    bq = _pmajor(bq)
    bres = _pmajor(bres)
    t2, u2 = _host_consts()
    x8 = np.stack([_pmajor(x[b].astype(f8)) for b in range(B)])
    x16 = np.stack([_pmajor(x[b].astype(ml_dtypes.bfloat16)) for b in range(B)])

    nc = _get_nc()
    in_maps = []
    for b in range(B):
        in_maps.append(
            {
                "x8": x8[b],
                "x16": x16[b],
                "wp": wp8,
                "wo": wo8,
                "bq": bq,
                "bres": bres,
                "t2": t2,
                "u2": u2,
            }
        )
    res = run_bass_kernel_spmd(nc, in_maps, core_ids=list(range(B)), **spmd_kwargs)
    h = w = int(np.sqrt(HW))
    out = np.stack([res.results[b]["out"].reshape(C, h, w) for b in range(B)])
    return out, res


def kernel(x, Wp, bp, Wo, bo):
    out, _ = run_sharded(x, Wp, bp, Wo, bo)
    return out


# revision 44
# speedup vs baseline: 1.0240x; 1.0240x over previous
"""AttentionBlock kernel for 8x Trainium2 NeuronCores.

Strategy: data-parallel over batch (B=8 -> 1 batch element per core), with
fp8-e4m3 DoubleRow matmuls (K=256 contraction per instruction = 2x the
FLOPs/column of bf16) everywhere the numerics allow, bf16 only for the
score matmuls (exp amplifies q/k quantization noise; fp8 scores fail the
2e-2 gate, bf16 scores measure ~1.3e-2 in simulation).

Per-core layout (channel-major, "transposed", no on-chip transposes):

  x8  [128, 2, 1024] fp8 pairs   (contraction rows c = i*256 + s*128 + p)
  wp8 [128, 2, 1536] fp8 pairs   (host-reordered columns [Q|K|V], x16 scale)
  A:  q/k^T[d, tok]  = wp8^T x8 (fp8 DR), DVE adds 16*bq to q (k-bias
      cancels in softmax and is dropped)
  B:  v[tok, och]    = x8^T wp8_V (fp8 DR), Pool copies psum -> v8 pairs
  S:  S^T[j, i]      = kT^T qT per j-tile (bf16)
  E = exp(S * scale/256 - 4) on ScalarE, fp8 out, pair tiles (the -4 bias
      keeps e^s under fp8-e4m3 max 240 and cancels in softmax)
  AV: O^T[d, i]     += v8^T E (fp8 DR); CS: colsums via banded-ones fp8 DR
      matmuls into a [2, 512] psum tile per head
  recip (DVE f32r), broadcast via u-band matmul (rows scaled 1/16), norm:
      Pool copies O psum -> bf16, DVE multiplies by broadcast recip -> fp8
  D:  out^T[c, tok]  = wo8^T oT8 (fp8 DR) + (bres + x) via one STT op,
      where bres = bo + Wo^T bv is precomputed on the host (sum_j P = 1).

Schedule: ScalarE runs the 32 exp ops back to back (~30us); the PE stream
is emitted so scores of head h+1 and AV/CS of head h interleave inside
head h's exp window.  All input DMAs trigger from the Pool queue (cheap
descriptor config), x8 from the SP queue, outputs from SP at the tail.
"""

import sys

sys.path.insert(0, "/opt/trn_rl_repo")

import numpy as np
import ml_dtypes

import concourse.bass as bass
import concourse.tile as tile
import concourse.mybir as mybir
from concourse.bass_utils import run_bass_kernel_spmd

B, C, HW = 8, 512, 1024
NH, DK = 4, 128
SCALE = float(DK) ** -0.5
WPS = 16.0  # host pre-scale of Wp into fp8 normal range
EXP_BIAS = -4.0

F32 = mybir.dt.float32
F32R = mybir.dt.float32r
BF16 = mybir.dt.bfloat16
F8 = mybir.dt.float8e4
DR = mybir.MatmulPerfMode.DoubleRow

# ---------------------------------------------------------------------------
# Walrus in this container supports only ONE embedded sync-wait per
# instruction; Tile emits multi-wait instructions, so rewrite each into
# single-wait NoOps + the instruction keeping its last wait.
# ---------------------------------------------------------------------------
_wsplit_counter = [0]


def _split_multi_waits(nc):
    for fn in nc.m.functions:
        for blk in fn.blocks:
            insts = blk.instructions
            if not insts:
                continue
            new = []
            changed = False
            for inst in insts:
                si = inst.sync_info
                waits = list(si.on_wait) if si is not None and si.on_wait else []
                if len(waits) > 1:
                    changed = True
                    for w in waits[:-1]:
                        _wsplit_counter[0] += 1
                        nop = mybir.InstNoOp(
                            name=f"WSPLIT-{_wsplit_counter[0]}",
                            ins=[],
                            outs=[],
                            engine=inst.engine,
                        )
                        nop.sync_info = mybir.SyncInfo(on_wait=[w], on_update=[])
                        nc.register_instruction(nop, overwrite=True)
                        new.append(nop)
                    inst.sync_info = mybir.SyncInfo(
                        on_wait=[waits[-1]], on_update=list(si.on_update or [])
                    )
                new.append(inst)
            if changed:
                blk.instructions = new


def build_attention_nc():
    nc = bass.Bass("TRN2")
    # partition-major host layouts: one contiguous DMA per tensor chunk
    x8d = nc.dram_tensor("x8", [128, 4 * HW], F8, kind="ExternalInput")
    x16d = nc.dram_tensor("x16", [128, 4 * HW], BF16, kind="ExternalInput")
    # wp block-major: [p, (block q/k/v, c-subtile, 512)]
    wpd = nc.dram_tensor("wp", [128, 3 * 4 * 512], F8, kind="ExternalInput")
    wod = nc.dram_tensor("wo", [128, 4 * C], F8, kind="ExternalInput")
    bqd = nc.dram_tensor("bq", [128, 4], F32, kind="ExternalInput")
    bresd = nc.dram_tensor("bres", [128, 4], F32, kind="ExternalInput")
    t2d = nc.dram_tensor("t2", [128, 512], F8, kind="ExternalInput")
    u2d = nc.dram_tensor("u2", [2, 256], F32R, kind="ExternalInput")
    outd = nc.dram_tensor("out", [C, HW], F32, kind="ExternalOutput")

    x8d, x16d, wpd, wod, bqd, bresd, t2d, u2d, outd = (
        t.ap() for t in (x8d, x16d, wpd, wod, bqd, bresd, t2d, u2d, outd)
    )

    EXP = mybir.ActivationFunctionType.Exp
    ADD = mybir.AluOpType.add
    MUL = mybir.AluOpType.mult
    IC = [slice(0, 512), slice(512, 1024)]

    with tile.TileContext(nc) as tc:
        with (
            tc.tile_pool(name="persist", bufs=1) as persist,
            tc.tile_pool(name="epool", bufs=8) as epool,
            tc.tile_pool(name="o16p", bufs=4) as o16p,
            tc.tile_pool(name="outp", bufs=2) as outp,
            tc.tile_pool(name="psM", bufs=2, space="PSUM") as psM,
            tc.tile_pool(name="psX", bufs=4, space="PSUM") as psX,
        ):
            # ---- persistent SBUF tensors -------------------------------
            x8_sb = persist.tile([128, 4 * HW], F8, tag="x8", name="x8")
            x16_sb = persist.tile([128, 4 * HW], BF16, tag="x16", name="x16")
            wp_sb = [
                persist.tile([128, 2048], F8, tag=f"wp{b}", name=f"wp{b}")
                for b in range(3)
            ]
            wo_sb = persist.tile([128, 4 * C], F8, tag="wo", name="wo")
            qk_sb = [persist.tile([128, HW], BF16, tag=f"qk{i}", name=f"qk{i}") for i in range(8)]
            v8_sb = [persist.tile([128, 1024], F8, tag=f"v{i}", name=f"v{i}") for i in range(4)]
            oT8_sb = [persist.tile([128, 2048], F8, tag=f"oT{i}", name=f"oT{i}") for i in range(2)]
            bq_sb = persist.tile([128, 4], F32, tag="bq", name="bq")
            bres_sb = persist.tile([128, 4], F32, tag="br", name="br")
            csr_sb = [persist.tile([2, 512], F32R, tag=f"cs{i}", name=f"cs{i}") for i in range(4)]
            t2_sb = persist.tile([128, 512], F8, tag="t2", name="t2")
            u2_sb = persist.tile([2, 256], F32R, tag="u2", name="u2")
            warm_sb = persist.tile([1, 2], F32, tag="warm", name="warm")
            ebias_sb = persist.tile([128, 1], F32, tag="ebias", name="ebias")

            x84 = x8_sb[:].rearrange("p (c n) -> p c n", c=4)
            # per-block [p, c-subtile, 512] views of wp
            wpb = [wp_sb[b][:].rearrange("p (c n) -> p c n", c=4) for b in range(3)]
            wo4 = wo_sb[:].rearrange("p (c n) -> p c n", c=4)
            v83 = [t[:].rearrange("p (s n) -> p s n", s=2) for t in v8_sb]
            oT83 = [t[:].rearrange("p (s n) -> p s n", s=2) for t in oT8_sb]
            t23 = [
                t2_sb[:, 0:256].rearrange("p (s n) -> p s n", s=2),
                t2_sb[:, 256:512].rearrange("p (s n) -> p s n", s=2),
            ]

            # ---- loads: few big DMAs (each completion sem on the
            # critical path costs ~1us; ring FIFO follows trigger order) --
            nc.sync.dma_start(out=x8_sb[:, 0:2048], in_=x8d[:, 0:2048])
            nc.sync.dma_start(out=x8_sb[:, 2048:4096], in_=x8d[:, 2048:4096])
            for b in range(3):
                nc.sync.dma_start(
                    out=wp_sb[b], in_=wpd[:, b * 2048 : (b + 1) * 2048]
                )
            nc.scalar.dma_start(out=bq_sb, in_=bqd[:, :])
            nc.scalar.dma_start(out=t2_sb, in_=t2d[:, :])
            nc.scalar.dma_start(out=u2_sb, in_=u2d[:, :])
            nc.scalar.dma_start(out=bres_sb, in_=bresd[:, :])
            # bulk needed only at the tail: the gate copy READS the x8
            # tile (already written above in program order), so these DMAs
            # fire only after the startup-critical x8 transfer completes
            # and stay off the rings until then
            dgate = persist.tile([1, 2], F8, tag="dgate", name="dgate")
            nc.gpsimd.tensor_copy(out=dgate[:], in_=x8_sb[0:1, 0:2])
            nc.gpsimd.dma_start(out=x16_sb[:, 0:2048], in_=x16d[:, 0:2048])
            nc.gpsimd.dma_start(out=x16_sb[:, 2048:4096], in_=x16d[:, 2048:4096])
            nc.gpsimd.dma_start(out=wo_sb, in_=wod[:, :])

            # ---- preload the Exp table while DMAs land -----------------
            nc.vector.memset(warm_sb[:], 0.0)
            nc.vector.memset(ebias_sb[:], EXP_BIAS)
            nc.scalar.activation(
                out=warm_sb[0:1, 0:1],
                in_=warm_sb[0:1, 1:2],
                func=EXP,
                bias=ebias_sb[0:1],
            )

            # ---- helpers ----------------------------------------------
            def proj_qk(h, qk):
                """q/k^T [128 d, 1024] per-ic psum halves (fp8 DR) + DVE cast."""
                for ic in range(2):
                    ps = psX.tile([128, 512], F32, tag="psX", name="psX")
                    for i in range(2):
                        nc.tensor.matmul(
                            ps[:],
                            wpb[qk][:, 2 * i : 2 * i + 2, h * 128 : (h + 1) * 128],
                            x84[:, 2 * i : 2 * i + 2, IC[ic]],
                            start=(i == 0),
                            stop=(i == 1),
                            perf_mode=DR,
                            skip_group_check=True,
                        )
                    if qk == 0:
                        nc.vector.tensor_scalar_add(
                            out=qk_sb[h * 2][:, IC[ic]],
                            in0=ps[:],
                            scalar1=bq_sb[:, h : h + 1],
                        )
                    else:
                        nc.vector.tensor_copy(out=qk_sb[h * 2 + 1][:, IC[ic]], in_=ps[:])

            def proj_v(jt):
                """v [tok, och] one j-tile (fp8 DR) + Act copy to fp8."""
                ps = psX.tile([128, 512], F32, tag="psX", name="psX")
                for i in range(2):
                    nc.tensor.matmul(
                        ps[:],
                        x84[:, 2 * i : 2 * i + 2, jt * 128 : (jt + 1) * 128],
                        wpb[2][:, 2 * i : 2 * i + 2, :],
                        start=(i == 0),
                        stop=(i == 1),
                        perf_mode=DR,
                        skip_group_check=True,
                    )
                nc.scalar.copy(
                    out=v8_sb[jt // 2][:, (jt % 2) * 512 : (jt % 2 + 1) * 512],
                    in_=ps[:],
                )

            e_tiles = {}

            def scores(h, jt):
                """S^T [128 j, 1024 i] (bf16) + exp -> E fp8 pair slot."""
                qT, kT = qk_sb[h * 2], qk_sb[h * 2 + 1]
                ps = psM.tile([128, HW], F32, tag="psM", name="psM")
                for ic in range(2):
                    nc.tensor.matmul(
                        ps[:, IC[ic]],
                        kT[:, jt * 128 : (jt + 1) * 128],
                        qT[:, IC[ic]],
                    )
                jp, s = jt // 2, jt % 2
                if s == 0:
                    e_tiles[(h, jp)] = epool.tile([128, 2048], F8, tag="E", name="E")
                nc.scalar.activation(
                    out=e_tiles[(h, jp)][:, s * 1024 : (s + 1) * 1024],
                    in_=ps[:],
                    func=EXP,
                    scale=SCALE / (WPS * WPS),
                    bias=ebias_sb[:],
                )

            def e3(h, jp):
                return e_tiles[(h, jp)][:].rearrange("p (s n) -> p s n", s=2)

            av_ps = {}  # (h, ic) -> psX tile
            cs_ps = {}  # h -> psX tile

            def av_alloc(h, ic):
                av_ps[(h, ic)] = psX.tile([128, 512], F32, tag="psX", name="psX")

            def av_mm(h, jp, ic):
                if (h, ic) not in av_ps:
                    av_alloc(h, ic)
                nc.tensor.matmul(
                    av_ps[(h, ic)][:],
                    v83[jp][:, :, h * 128 : (h + 1) * 128],
                    e3(h, jp)[:, :, IC[ic]],
                    start=(jp == 0),
                    stop=(jp == 3),
                    perf_mode=DR,
                    skip_group_check=True,
                )

            def cs_mm(h, jp):
                if jp == 0:
                    cs_ps[h] = psX.tile([128, 512], F32, tag="psX", name="psX")
                for ic in range(2):
                    nc.tensor.matmul(
                        cs_ps[h][:],
                        t23[ic],
                        e3(h, jp)[:, :, IC[ic]],
                        start=(jp == 0 and ic == 0),
                        stop=(jp == 3 and ic == 1),
                        perf_mode=DR,
                        skip_group_check=True,
                    )

            o16_tiles = {}

            def o_copy(h, ic):
                o16 = o16p.tile([128, 512], BF16, tag="o16", name="o16")
                nc.vector.tensor_copy(out=o16[:], in_=av_ps[(h, ic)][:])
                o16_tiles[(h, ic)] = o16

            def recip(h):
                with nc.allow_low_precision(reason="softmax denom recip f32r"):
                    nc.vector.reciprocal(out=csr_sb[h][:], in_=cs_ps[h][0:2, :])

            def bc_norm(h, ic):
                bc = psX.tile([128, 512], F32, tag="psX", name="psX")
                nc.tensor.matmul(
                    bc[:],
                    u2_sb[:, (1 - ic) * 128 : (2 - ic) * 128],
                    csr_sb[h][:],
                )
                nc.vector.tensor_tensor(
                    out=oT8_sb[h // 2][
                        :, (h % 2) * 1024 + ic * 512 : (h % 2) * 1024 + (ic + 1) * 512
                    ],
                    in0=o16_tiles[(h, ic)][:],
                    in1=bc[:],
                    op=MUL,
                )

            d_ps = {}

            def proj_out_mm(kc, mlist):
                if kc not in d_ps:
                    d_ps[kc] = psM.tile([128, HW], F32, tag="psM", name="psM")
                ps = d_ps[kc]
                for m in mlist:
                    for ic in range(2):
                        nc.tensor.matmul(
                            ps[:, IC[ic]],
                            wo4[:, 2 * m : 2 * m + 2, kc * 128 : (kc + 1) * 128],
                            oT83[m][:, :, IC[ic]],
                            start=(m == 0),
                            stop=(m == 1),
                            perf_mode=DR,
                            skip_group_check=True,
                        )

            def proj_out_head(kc, h, stop):
                """single-head (K=128, non-DR fp8) contribution to d_ps[kc]."""
                ps = d_ps[kc]
                for ic in range(2):
                    nc.tensor.matmul(
                        ps[:, IC[ic]],
                        wo_sb[:, h * 512 + kc * 128 : h * 512 + (kc + 1) * 128],
                        oT8_sb[h // 2][
                            :, (h % 2) * 1024 + ic * 512 : (h % 2) * 1024 + (ic + 1) * 512
                        ],
                        start=False,
                        stop=stop,
                        skip_group_check=True,
                    )

            def finish(kc):
                ps = d_ps[kc]
                ot = outp.tile([128, HW], F32, tag="out", name="out")
                nc.vector.scalar_tensor_tensor(
                    out=ot[:],
                    in0=ps[:],
                    scalar=bres_sb[:, kc : kc + 1],
                    in1=x16_sb[:, kc * 1024 : (kc + 1) * 1024],
                    op0=ADD,
                    op1=ADD,
                )
                nc.sync.dma_start(out=outd[kc * 128 : (kc + 1) * 128, :], in_=ot[:])

            # ---- prologue ---------------------------------------------
            proj_qk(0, 0)
            proj_qk(0, 1)
            scores(0, 0)
            scores(0, 1)
            proj_qk(1, 0)
            scores(0, 2)
            scores(0, 3)
            proj_qk(1, 1)
            proj_v(0)
            proj_v(1)
            scores(0, 4)
            scores(0, 5)
            proj_v(2)
            proj_v(3)
            scores(0, 6)
            scores(0, 7)
            proj_v(4)
            proj_v(5)
            proj_v(6)
            proj_v(7)

            # ---- head loop --------------------------------------------
            # trailing ops of head h-1 (which wait the last exps of head
            # h-1) are emitted at the START of iter h so they never block
            # the next head's score stream; the psX 4-slot rotation order
            # is [AVic0, AVic1, CS] + bc pairs, arranged so every slot
            # reuse waits on a fast o_copy/cast, never the slow recip.
            for h in range(NH):
                if h < 3:
                    scores(h + 1, 0)
                    scores(h + 1, 1)
                if h >= 1:
                    cs_mm(h - 1, 3)
                    av_mm(h - 1, 3, 1)
                    o_copy(h - 1, 1)
                    recip(h - 1)
                if h == 0:
                    proj_qk(2, 0)
                    proj_qk(2, 1)
                if h < 3:
                    scores(h + 1, 2)
                    scores(h + 1, 3)
                if h == 0:
                    proj_qk(3, 0)
                    proj_qk(3, 1)
                if h >= 2:
                    bc_norm(h - 2, 0)
                    bc_norm(h - 2, 1)
                av_mm(h, 0, 0)
                av_alloc(h, 1)
                av_mm(h, 1, 0)
                cs_mm(h, 0)
                cs_mm(h, 1)
                if h < 3:
                    scores(h + 1, 4)
                    scores(h + 1, 5)
                cs_mm(h, 2)
                av_mm(h, 2, 0)
                av_mm(h, 0, 1)
                av_mm(h, 1, 1)
                if h == 3:
                    proj_out_mm(0, [0])
                    proj_out_mm(1, [0])
                if h < 3:
                    scores(h + 1, 6)
                    scores(h + 1, 7)
                av_mm(h, 3, 0)
                av_mm(h, 2, 1)
                o_copy(h, 0)

            # ---- tail -------------------------------------------------
            cs_mm(3, 3)
            av_mm(3, 3, 1)
            o_copy(3, 1)
            recip(3)
            bc_norm(2, 0)
            bc_norm(2, 1)
            proj_out_head(0, 2, stop=False)
            proj_out_head(1, 2, stop=False)
            bc_norm(3, 0)
            bc_norm(3, 1)
            proj_out_head(0, 3, stop=True)
            proj_out_head(1, 3, stop=True)
            finish(0)
            finish(1)
            proj_out_mm(2, [0, 1])
            finish(2)
            proj_out_mm(3, [0, 1])
            finish(3)

    _split_multi_waits(nc)
    return nc


_NC_CACHE = {}


def _get_nc():
    if "nc" not in _NC_CACHE:
        _NC_CACHE["nc"] = build_attention_nc()
    return _NC_CACHE["nc"]


def _host_consts():
    f8 = ml_dtypes.float8_e4m3
    # selector weights [p, (half, s, 128)]: half ic has its column ic set,
    # in both k-subtile slots, so matmul output row ic gets the colsum
    t2 = np.zeros((128, 512), dtype=f8)
    for s in range(2):
        t2[:, s * 128 + 0] = 1.0
        t2[:, 256 + s * 128 + 1] = 1.0
    u2 = np.zeros((2, 256), dtype=np.float32)
    u2[0, 128:256] = 1.0 / WPS
    u2[1, 0:128] = 1.0 / WPS
    return t2, u2


def _prep_weights(Wp, bp, Wo, bo):
    """Reorder Wp/bp columns to [Q|K|V] blocks, pre-scale by WPS, cast fp8;
    fold the v bias through the output projection on the host."""
    f8 = ml_dtypes.float8_e4m3
    Wp = np.ascontiguousarray(Wp, dtype=np.float32)
    bp = np.ascontiguousarray(bp, dtype=np.float32).reshape(-1)
    Wo = np.ascontiguousarray(Wo, dtype=np.float32)
    bo = np.ascontiguousarray(bo, dtype=np.float32).reshape(-1)
    qcols = np.concatenate([np.arange(h * 384, h * 384 + 128) for h in range(NH)])
    kcols = qcols + 128
    vcols = qcols + 256
    order = np.concatenate([qcols, kcols, vcols])
    wp8 = (Wp[:, order] * WPS).astype(f8)
    bq = (bp[qcols] * WPS).astype(np.float32).reshape(C, 1)
    bres = (bo + Wo.T @ bp[vcols]).astype(np.float32).reshape(C, 1)
    wo8 = Wo.astype(f8)
    return wp8, bq, wo8, bres


def _pmajor(a):
    """[512, N] -> [128, 4*N]: row c = cs*128 + p lands at [p, cs*N + n]."""
    n = a.shape[1]
    return np.ascontiguousarray(a.reshape(4, 128, n).transpose(1, 0, 2).reshape(128, 4 * n))


def run_sharded(x, Wp, bp, Wo, bo, **spmd_kwargs):
    """Shard over batch, run on cores 0-7, gather.  Returns ([B,C,H,W], res)."""
    f8 = ml_dtypes.float8_e4m3
    x = np.ascontiguousarray(x, dtype=np.float32).reshape(B, C, HW)
    wp8, bq, wo8, bres = _prep_weights(Wp, bp, Wo, bo)
    # block-major: [p, (block, c-subtile, 512)]
    wp8 = np.ascontiguousarray(
        wp8.reshape(4, 128, 3, 512).transpose(1, 2, 0, 3).reshape(128, 6144)
    )
    wo8 = _pmajor(wo8)
    bq = _pmajor(bq)
    bres = _pmajor(bres)
    t2, u2 = _host_consts()
    x8 = np.stack([_pmajor(x[b].astype(f8)) for b in range(B)])
    x16 = np.stack([_pmajor(x[b].astype(ml_dtypes.bfloat16)) for b in range(B)])

    nc = _get_nc()
    in_maps = []
    for b in range(B):
        in_maps.append(
            {
                "x8": x8[b],
                "x16": x16[b],
                "wp": wp8,
                "wo": wo8,
                "bq": bq,
                "bres": bres,
                "t2": t2,
                "u2": u2,
            }
        )
    res = run_bass_kernel_spmd(nc, in_maps, core_ids=list(range(B)), **spmd_kwargs)
    h = w = int(np.sqrt(HW))
    out = np.stack([res.results[b]["out"].reshape(C, h, w) for b in range(B)])
    return out, res


def kernel(x, Wp, bp, Wo, bo):
    out, _ = run_sharded(x, Wp, bp, Wo, bo)
    return out
